# revision 1
# baseline (speedup 1.0000x reference)
"""Trainium2 Bass kernel for nn_Attention_63513976373985.

Strategy: pure data-parallel over the batch dim B=64 across 8 NeuronCores
(8 batches per core, all params replicated, no collectives). Inside each
core, per-batch pipeline:
  X = d2[b]            [S=512, F=512]  fp16 in DRAM, loaded TRANSPOSED via
                       the xbar DMA-transpose (2-byte dtypes only)
  d3T = relu(w1.T @ XT + b1)           [C, S]   (fp16 matmul, f32 PSUM)
  tv  = tanh(XT.T @ wv)                [S, C]   natural layout for vs
  per head h: zsT = Wtop[h].T @ d3T  (+ zconst[h,b] per-partition ACT bias,
              where zconst = relu(d1@w1+b1) @ Wbot[h] — the d4 half of d5)
              usT = tanh(zsT + zconst)           [C, S]
  atts = blockdiag(P) matvec over usT            [H, S]
  softmax over S (ACT exp w/ accum_out, DVE reciprocal; 1/Z folded into
  the vs eviction as a per-partition scale)
  scoresT via PE transpose, vs = scoresT.T @ tv  [H, C]
  V slabs via PE transpose, out = relu(V.T @ wcc + bcc)  [BLOC, 128]

Everything runs fp16 (same 10-bit mantissa as tf32; fp32 PSUM
accumulate). No float32r anywhere: f32r DMA loads engage a rounding mode
that corrupts concurrently-executing fp16 xbar DMA-transposes on this HW.
"""
import sys

if "/opt/trn_rl_repo" not in sys.path:
    sys.path.insert(0, "/opt/trn_rl_repo")

import numpy as np

H, F, C, S, B = 8, 512, 256, 512, 64
NCORES = 8
BLOC = B // NCORES  # 8
OUTF = 128

_CACHE = {}


def build_nc():
    import concourse.bass as bass  # noqa: F401
    import concourse.mybir as mybir
    import concourse.tile as tile
    from concourse import bacc
    from contextlib import ExitStack

    f32 = mybir.dt.float32
    f16 = mybir.dt.float16
    AF = mybir.ActivationFunctionType

    nc = bacc.Bacc("TRN2", target_bir_lowering=False, debug=False,
                   num_devices=NCORES)

    # ---- DRAM parameters (per-core shard shapes) ----
    d2_d = nc.dram_tensor("d2", [BLOC, S, F], f16, kind="ExternalInput")
    d1t_d = nc.dram_tensor("d1t", [128, 4, BLOC], f16, kind="ExternalInput")
    w1_d = nc.dram_tensor("w1r", [128, 4, 2, 128], f16, kind="ExternalInput")
    wv_d = nc.dram_tensor("wvr", [128, 4, C], f16, kind="ExternalInput")
    wtop_d = nc.dram_tensor("wtopr", [128, H, 2, 2, 128], f16, kind="ExternalInput")
    wbot_d = nc.dram_tensor("wbotr", [128, H, 2, 2, 128], f16, kind="ExternalInput")
    pblk_d = nc.dram_tensor("pblkr", [128, 2 * H, H], f16, kind="ExternalInput")
    wcc_d = nc.dram_tensor("wccr", [128, 2 * H, OUTF], f16, kind="ExternalInput")
    bcc_d = nc.dram_tensor("bccr", [1, OUTF], f16, kind="ExternalInput")
    b1c_d = nc.dram_tensor("b1c", [128, 2], f32, kind="ExternalInput")
    id8_d = nc.dram_tensor("id8", [8, 8], f16, kind="ExternalInput")
    ones18_d = nc.dram_tensor("ones18", [1, 8], f16, kind="ExternalInput")
    out_d = nc.dram_tensor("out", [BLOC, OUTF], f32, kind="ExternalOutput")

    with tile.TileContext(nc) as tc, ExitStack() as stk:
        const = stk.enter_context(tc.tile_pool(name="const", bufs=1))
        xtp = stk.enter_context(tc.tile_pool(name="xtp", bufs=3))
        d3p = stk.enter_context(tc.tile_pool(name="d3p", bufs=3))
        tvpool = stk.enter_context(tc.tile_pool(name="tvpool", bufs=2))
        usp = stk.enter_context(tc.tile_pool(name="usp", bufs=4))
        smallsb = stk.enter_context(tc.tile_pool(name="smallsb", bufs=2))
        vpool = stk.enter_context(tc.tile_pool(name="vpool", bufs=1))
        pmm = stk.enter_context(tc.tile_pool(name="pmm", bufs=2, space="PSUM"))
        pzs = stk.enter_context(tc.tile_pool(name="pzs", bufs=3, space="PSUM"))
        patp = stk.enter_context(tc.tile_pool(name="patp", bufs=2, space="PSUM"))
        psmall = stk.enter_context(
            tc.tile_pool(name="psmall", bufs=1, space="PSUM"))

        # ---- constants + X prefetch, ordered so PE can start ASAP ----

        # XT loads: [f(4 tiles of 128), s=512] via the xbar DMA transpose
        # (sync queue only: DMA_TRANSPOSE occupies its issuing queue ~1.3us,
        # putting any on nc.scalar stalls the ACT tanh stream)
        def load_xt(b, name):
            xt = xtp.tile([128, 4, S], f16, tag="xt", name=name)
            for kf in range(4):
                nc.sync.dma_start_transpose(
                    out=xt[:, kf, :],
                    in_=d2_d[b, :, kf * 128:(kf + 1) * 128])
            return xt

        id8_sb = const.tile([8, 8], f16, tag="id8")
        nc.sync.dma_start(out=id8_sb, in_=id8_d[:, :])
        w1_sb = const.tile([128, 4, 2, 128], f16, tag="w1")
        nc.sync.dma_start(out=w1_sb, in_=w1_d[:, :, :, :])
        d1t_sb = const.tile([128, 4, BLOC], f16, tag="d1t")
        nc.sync.dma_start(out=d1t_sb, in_=d1t_d[:, :, :])
        b1c_sb = const.tile([128, 2], f32, tag="b1c")
        nc.sync.dma_start(out=b1c_sb, in_=b1c_d[:, :])
        wv_sb = const.tile([128, 4, C], f16, tag="wv")
        nc.sync.dma_start(out=wv_sb, in_=wv_d[:, :, :])
        ones18_sb = const.tile([1, 8], f16, tag="ones18")
        nc.sync.dma_start(out=ones18_sb, in_=ones18_d[:, :])
        bcc_sb = const.tile([1, OUTF], f16, tag="bcc")
        nc.sync.dma_start(out=bcc_sb, in_=bcc_d[:, :])

        # prefetch b=0 XT before the heavy weight DMAs
        xt0 = load_xt(0, "xt_pre0")

        wbot_sb = const.tile([128, H, 2, 2, 128], f16, tag="wbot")
        for h in range(H):
            nc.sync.dma_start(out=wbot_sb[:, h, :, :, :],
                              in_=wbot_d[:, h, :, :, :])
        pblk_sb = const.tile([128, 2 * H, H], f16, tag="pblk")
        nc.sync.dma_start(out=pblk_sb, in_=pblk_d[:, :, :])
        wtop_sb = const.tile([128, H, 2, 2, 128], f16, tag="wtop")
        for h in range(H):
            nc.sync.dma_start(out=wtop_sb[:, h, :, :, :],
                              in_=wtop_d[:, h, :, :, :])
        wcc_sb = const.tile([128, 2 * H, OUTF], f16, tag="wcc")

        # ---- d4T = relu(w1.T @ d1T + b1) : [C(2 tiles), BLOC] ----
        pd4 = psmall.tile([128, 2, BLOC], f32, tag="small")
        for m in range(2):
            for k in range(4):
                nc.tensor.matmul(pd4[:, m, :], lhsT=w1_sb[:, k, m, :],
                                 rhs=d1t_sb[:, k, :],
                                 start=(k == 0), stop=(k == 3))
        d4t_sb = const.tile([128, 2, BLOC], f16, tag="d4t")
        for m in range(2):
            nc.scalar.activation(d4t_sb[:, m, :], pd4[:, m, :], AF.Relu,
                                 bias=b1c_sb[:, m:m + 1])

        # ---- zconstT[h] = Wbot[h].T @ d4T : [2, C-tile, h, b] layout ----
        pzc = psmall.tile([128, 2, H, BLOC], f32, tag="small")
        for ct in range(2):
            for h in range(H):
                for ks in range(2):
                    nc.tensor.matmul(pzc[:, ct, h, :],
                                     lhsT=wbot_sb[:, h, ks, ct, :],
                                     rhs=d4t_sb[:, ks, :],
                                     start=(ks == 0), stop=(ks == 1))
        zc_sb = const.tile([128, 2, H, BLOC], f32, tag="zc")
        nc.vector.tensor_copy(out=zc_sb, in_=pzc)

        # ---- V accumulator across the b loop ----
        v_sb = vpool.tile([128, 2, H, BLOC], f16)  # [c-in-half, ch, h, b]

        for b in range(BLOC):
            # 1) XT via DMA transpose (b=0 prefetched above)
            xt = xt0 if b == 0 else load_xt(b, f"xt{b}")
            if b == 2:
                # wcc only needed for the final projection; load mid-stream
                nc.sync.dma_start(out=wcc_sb, in_=wcc_d[:, :, :])
            # 3) d3T = relu(w1.T @ XT + b1) [C(2), S]  (fp16 matmul)
            d3t = d3p.tile([128, 2, S], f16, tag="d3t", name=f"d3t{b}")
            for m in range(2):
                pmd3 = pmm.tile([128, S], f32, tag="mm", name=f"pmd3_{b}_{m}")
                for kf in range(4):
                    nc.tensor.matmul(pmd3, lhsT=w1_sb[:, kf, m, :],
                                     rhs=xt[:, kf, :],
                                     start=(kf == 0), stop=(kf == 3))
                nc.scalar.activation(d3t[:, m, :], pmd3, AF.Relu,
                                     bias=b1c_sb[:, m:m + 1])
            # 4) tv = tanh(X @ wv) [S(4), C] natural; 2 M-tiles per PSUM bank
            tv = tvpool.tile([128, 4, C], f16, tag="tv", name=f"tv{b}")
            for mp in range(2):
                pmtv = pmm.tile([128, 2, C], f32, tag="mm",
                                name=f"pmtv_{b}_{mp}")
                for ms2 in range(2):
                    ms = mp * 2 + ms2
                    for kf in range(4):
                        nc.tensor.matmul(
                            pmtv[:, ms2, :],
                            lhsT=xt[:, kf, ms * 128:(ms + 1) * 128],
                            rhs=wv_sb[:, kf, :],
                            start=(kf == 0), stop=(kf == 3))
                nc.scalar.activation(tv[:, mp * 2:(mp + 1) * 2, :], pmtv,
                                     AF.Tanh)
            # 5) per-head zs/us + atts accumulate
            pat = patp.tile([8, S], f32, tag="atts", name=f"pat{b}")
            for h in range(H):
                us = usp.tile([128, 2, S], f16, tag="us", name=f"us{b}_{h}")
                for ct in range(2):
                    pz = pzs.tile([128, S], f32, tag="zs",
                                  name=f"pz{b}_{h}_{ct}")
                    for ks in range(2):
                        nc.tensor.matmul(pz, lhsT=wtop_sb[:, h, ks, ct, :],
                                         rhs=d3t[:, ks, :],
                                         start=(ks == 0), stop=(ks == 1))
                    nc.scalar.activation(us[:, ct, :], pz, AF.Tanh,
                                         bias=zc_sb[:, ct, h, b:b + 1])
                for ct in range(2):
                    nc.tensor.matmul(pat, lhsT=pblk_sb[:, h * 2 + ct, :],
                                     rhs=us[:, ct, :],
                                     start=(h == 0 and ct == 0),
                                     stop=(h == H - 1 and ct == 1))
            # 6) softmax over S (normalization deferred to the vs eviction)
            nmax = smallsb.tile([8, 1], f32, tag="nmax", name=f"nmax{b}")
            nc.vector.tensor_reduce(nmax, pat, axis=mybir.AxisListType.X,
                                    op=mybir.AluOpType.max, negate=True)
            esc = smallsb.tile([8, S], f16, tag="esc", name=f"esc{b}")
            zsum = smallsb.tile([8, 1], f32, tag="zsum", name=f"zsum{b}")
            nc.scalar.activation(esc, pat, AF.Exp, bias=nmax, accum_out=zsum)
            zinv = smallsb.tile([8, 1], f32, tag="zinv", name=f"zinv{b}")
            nc.vector.reciprocal(zinv, zsum)
            # 7) scoresT (unnormalized) via PE transpose: [S(4 tiles), 8]
            psc = psmall.tile([128, 4, 8], f16, tag="small", name=f"psc{b}")
            for sc in range(4):
                nc.tensor.transpose(psc[:, sc, :],
                                    in_=esc[:, sc * 128:(sc + 1) * 128],
                                    identity=id8_sb)
            sct = smallsb.tile([128, 4, 8], f16, tag="sct", name=f"sct{b}")
            nc.vector.tensor_copy(out=sct, in_=psc)
            # 8) vs = scoresT.T @ tv : [8, C]; 1/Z applied at eviction
            pvs = psmall.tile([8, C], f32, tag="small", name=f"pvs{b}")
            for sc in range(4):
                nc.tensor.matmul(pvs, lhsT=sct[:, sc, :],
                                 rhs=tv[:, sc, :],
                                 start=(sc == 0), stop=(sc == 3))
            vssb = smallsb.tile([8, C], f16, tag="vssb", name=f"vssb{b}")
            nc.vector.tensor_scalar_mul(vssb, pvs, zinv)
            # 9) vsT into V slabs
            pvt = psmall.tile([128, 2, 8], f16, tag="small", name=f"pvt{b}")
            for ch in range(2):
                nc.tensor.transpose(pvt[:, ch, :],
                                    in_=vssb[:, ch * 128:(ch + 1) * 128],
                                    identity=id8_sb)
            for ch in range(2):
                nc.vector.tensor_copy(out=v_sb[:, ch, :, b:b + 1],
                                      in_=pvt[:, ch, :])

        # ---- final: out = relu(V.T @ wcc + bcc) ----
        pout = psmall.tile([8, OUTF], f32, tag="small")
        kidx = 0
        for h in range(H):
            for ch in range(2):
                nc.tensor.matmul(pout, lhsT=v_sb[:, ch, h, :],
                                 rhs=wcc_sb[:, h * 2 + ch, :],
                                 start=(kidx == 0), stop=False)
                kidx += 1
        nc.tensor.matmul(pout, lhsT=ones18_sb, rhs=bcc_sb,
                         start=False, stop=True)
        outsb = smallsb.tile([8, OUTF], f32, tag="outsb")
        nc.scalar.activation(outsb, pout, AF.Relu)
        nc.sync.dma_start(out=out_d[:, :], in_=outsb)

    nc.compile()
    return nc


def host_inputs(d1, d2, w1, b1, W, P, wv, wcc, bcc):
    """Host-side sharding + layout prep. Returns in_maps for 8 cores."""
    d1 = np.ascontiguousarray(d1, dtype=np.float32)
    d2 = np.ascontiguousarray(d2, dtype=np.float32)
    w1 = np.ascontiguousarray(w1, dtype=np.float32)
    b1 = np.ascontiguousarray(b1, dtype=np.float32)
    W = np.ascontiguousarray(W, dtype=np.float32)
    P = np.ascontiguousarray(P, dtype=np.float32)
    wv = np.ascontiguousarray(wv, dtype=np.float32)
    wcc = np.ascontiguousarray(wcc, dtype=np.float32)
    bcc = np.ascontiguousarray(bcc, dtype=np.float32)

    w1r = np.ascontiguousarray(
        w1.reshape(4, 128, 2, 128).transpose(1, 0, 2, 3))
    wvr = np.ascontiguousarray(wv.reshape(4, 128, C).transpose(1, 0, 2))
    wtopr = np.ascontiguousarray(
        W[:, :C, :].reshape(H, 2, 128, 2, 128).transpose(2, 0, 1, 3, 4))
    wbotr = np.ascontiguousarray(
        W[:, C:, :].reshape(H, 2, 128, 2, 128).transpose(2, 0, 1, 3, 4))
    pblkr = np.zeros((128, 2 * H, H), np.float32)
    for h in range(H):
        for ct in range(2):
            pblkr[:, h * 2 + ct, h] = P[h, ct * 128:(ct + 1) * 128]
    wccr = np.ascontiguousarray(
        wcc.reshape(2 * H, 128, OUTF).transpose(1, 0, 2))
    bccr = np.ascontiguousarray(bcc[None, :])
    b1c = np.ascontiguousarray(b1.reshape(2, 128).T)
    id8 = np.eye(8, dtype=np.float32)
    ones18 = np.ones((1, 8), np.float32)

    f16 = np.float16
    shared = dict(w1r=w1r.astype(f16), wvr=wvr.astype(f16),
                  wtopr=wtopr.astype(f16), wbotr=wbotr.astype(f16),
                  pblkr=pblkr.astype(f16), wccr=wccr.astype(f16),
                  bccr=bccr.astype(f16), b1c=b1c, id8=id8.astype(f16),
                  ones18=ones18.astype(f16))
    in_maps = []
    for core in range(NCORES):
        bs = slice(core * BLOC, (core + 1) * BLOC)
        d2c = np.ascontiguousarray(
            d2[:, bs, :].transpose(1, 0, 2).astype(np.float16))
        d1c = d1[bs]  # [BLOC, F]
        d1tr = np.ascontiguousarray(
            d1c.T.reshape(4, 128, BLOC).transpose(1, 0, 2)).astype(np.float16)
        in_maps.append(dict(d2=d2c, d1t=d1tr, **shared))
    return in_maps


def kernel(**inputs):
    if "nc" not in _CACHE:
        _CACHE["nc"] = build_nc()
    nc = _CACHE["nc"]
    in_maps = host_inputs(
        d1=inputs["d1"], d2=inputs["d2"], w1=inputs["w1"], b1=inputs["b1"],
        W=inputs["W"], P=inputs["P"], wv=inputs["wv"], wcc=inputs["wcc"],
        bcc=inputs["bcc"])
    from concourse.bass_utils import run_bass_kernel_spmd
    res = run_bass_kernel_spmd(nc, in_maps, core_ids=list(range(NCORES)))
    return np.concatenate([res.results[i]["out"] for i in range(NCORES)],
                          axis=0)

